# revision 1
# baseline (speedup 1.0000x reference)
"""Trainium2 Bass kernel for nn_CrossAttention_28183575396415.

The reference block-mask gives every query exactly one key (kv = q_idx // 3),
so the softmax weight is identically 1 and the q/k projections, RMSNorm and
RoPE are dead code.  The module reduces to

    out[b, t] = x_kv[b, t // 3] @ Wv.T @ Wproj.T
              = x_kv[b, t // 3] @ WfT          with WfT = Wv.T @ Wproj.T

Strategy (8 NeuronCores, SPMD):
  - Host folds the two projection matrices into WfT (computed in float64,
    stored float32) — constant folding of adjacent linear layers.
  - The 4*2048 = 8192 kv rows are row-sharded 8 ways (1024 rows/core).
    Each core's shard is pre-transposed on host so every device DMA is a
    natural contiguous load and the PE needs no on-device transposes; the
    shard and the weight are concatenated into one [1024(k), 2048] input so
    each k-tile arrives in a single 1 MiB DMA:
        xw[:, :1024]  = x_shard.T   (k on partitions = contraction dim)
        xw[:, 1024:]  = WfT
  - Device: z = xT.T @ WfT with K accumulated in PSUM (8 k-tiles), then each
    z row tile is written to HBM three times (the t//3 replication), giving
    this core's contiguous [3072, 1024] slice of the flattened output.
  - Host unshard = concatenate the 8 slices.
"""

import json
import os

import numpy as np

import concourse.bass as bass
import concourse.mybir as mybir
from concourse.tile import TileContext
from concourse.vector_clock import ScopedClock
from concourse.bass_utils import run_bass_kernel_spmd

P = 128          # partitions
C = 1024         # model dim
K_T = C // P     # k tiles
M_T = C // P     # row tiles per core shard
N = 512          # matmul free dim (one PSUM bank of fp32)
L = 3            # replication factor (Tq // Tkv)
ROWS_PER_CORE = 1024
N_CORES = 8

# compute dtype: "f32r" (full-rate fp32 PE mode), "bf16", or "f32" (4x slower)
COMPUTE_DT = os.environ.get("KERNEL_COMPUTE_DT", "f32r")
# "device3": device writes the replicated [3072, 1024] slice (default)
# "host1":   device writes [1024, 1024]; host repeats rows (debug/compare only)
OUT_MODE = os.environ.get("KERNEL_OUT_MODE", "device3")


class SlimTailTileContext(TileContext):
    """Tile's kernel tail is drain -> barrier -> ~280 serialized per-semaphore
    clear instructions -> barrier (~8 us measured).  The clears only matter if
    the loaded NEFF executes more than once; every kernel() call here builds a
    fresh jit executable (fresh NEFF load, semaphores re-initialized), so skip
    them and the second barrier.  The drain still waits for every DMA queue,
    so outputs are complete before the program ends."""

    def _drain_and_barrier(self, tick_clock, wait_clock):
        # The SP drain (with its hoisted wait chain) already gates on every
        # engine's clock and every DMA queue, so outputs are complete when SP
        # retires; with no sem-clears to order, the closing all-engine
        # barrier adds nothing but latency.
        drain_inst = self.nc.sync.drain()
        wait_clock.add_sem_waits(
            drain_inst.ins, ScopedClock({None: tick_clock.global_clock})
        )
        popped = self.nc._tile_sem_poison_stack.pop()
        assert popped is self._sem_poison


def _split_multiwaits(nc: bass.Bass) -> None:
    """This container's walrus allows only ONE sync-wait on several
    instruction formats (Drain/CTRL, Matmult's LDWEIGHTS half, ...).  Tile
    can emit more.  Post-pass the serialized BIR: for any instruction with
    >1 on_wait, hoist all but the last wait onto single-wait EventSemaphore
    carriers inserted immediately before it on the same engine (waits then
    execute in queue order — semantics unchanged).  The patched JSON is
    pinned on the instance so every downstream serialization sees it."""
    raw = bass.Bass.to_json_bytes(nc)
    j = json.loads(raw)
    n_hoisted = 0
    for f in j["functions"]:
        for bb in f["blocks"]:
            new_insts = []
            for ins in bb["instructions"]:
                si = ins.get("sync_info")
                waits = si.get("on_wait", []) if si else []
                if len(waits) > 1:
                    for i, w in enumerate(waits[:-1]):
                        carrier = {
                            "engine": ins["engine"],
                            "ins": [],
                            "outs": [],
                            "name": f"{ins['name']}_hw{i}",
                            "opcode": "EventSemaphore",
                            "sync_info": {"on_update": [], "on_wait": [w]},
                        }
                        if "debug" in ins:
                            carrier["debug"] = ins["debug"]
                        new_insts.append(carrier)
                        n_hoisted += 1
                    si["on_wait"] = waits[-1:]
                new_insts.append(ins)
            bb["instructions"] = new_insts
    patched = json.dumps(j).encode()
    nc.to_json_bytes = lambda: patched


def _build(compute_dt: str, out_mode: str) -> bass.Bass:
    nc = bass.Bass("TRN2")
    in_mydt = {
        "bf16": mybir.dt.bfloat16,
        "f32r": mybir.dt.float32r,
        "f32": mybir.dt.float32,
    }[compute_dt]

    W2 = ROWS_PER_CORE + C  # concatenated [x | w] free dim
    xw = nc.dram_tensor("xw", [C, W2], in_mydt, kind="ExternalInput")
    n_rep = L if out_mode == "device3" else 1
    out = nc.dram_tensor(
        "out", [n_rep * ROWS_PER_CORE, C], mybir.dt.float32, kind="ExternalOutput"
    )

    xw_t = xw.rearrange("(t p) m -> t p m", p=P)  # [8, 128, 2048]
    # out row (n_rep*g + r) <- z row g
    out_rep = out.rearrange("(g r) c -> g r c", r=n_rep)  # [1024, n_rep, 1024]

    with SlimTailTileContext(nc) as tc:
        with (
            tc.tile_pool(name="xw", bufs=1) as xw_pool,
            tc.tile_pool(name="psum", bufs=8, space="PSUM") as psum_pool,
            tc.tile_pool(name="zout", bufs=6) as z_pool,
        ):
            # Load two k-tiles per DMA (2 MiB each, side by side in the free
            # dim) and alternate the trigger engine so the input stream isn't
            # paced by a single engine's ~1us-per-trigger issue cost.
            # First k-tile alone (1 MiB) so the PE can start as early as
            # possible; the rest in 2 MiB pair-DMAs to amortize trigger cost.
            in_eng = [nc.sync, nc.scalar]
            groups = [[0], [1, 2], [3, 4], [5, 6], [7]]
            xwk = [None] * K_T
            for j, grp in enumerate(groups):
                n = len(grp)
                t = xw_pool.tile([P, n * W2], in_mydt, name=f"xwp{j}", tag=f"xwp{j}")
                src = xw[grp[0] * P : (grp[0] + n) * P, :].rearrange(
                    "(g p) m -> p g m", p=P
                )
                dst = t[:].rearrange("p (g m) -> p g m", g=n)
                in_eng[j % 2].dma_start(dst, src)
                for i, k in enumerate(grp):
                    xwk[k] = (t, i * W2)

            # Two passes over the output-column halves.  Each pass keeps one
            # PSUM bank per row-tile (8 banks), accumulates over k in lockstep
            # with the input DMA stream, and its evictions/stores start right
            # after the last input byte — so the output DMA stream begins as
            # early as the data dependency allows and the two passes keep the
            # DMA engines saturated end-to-end.
            evict_eng = [
                lambda dst, src: nc.vector.tensor_copy(dst, src),
                lambda dst, src: nc.vector.tensor_copy(dst, src),
            ]
            out_eng = [nc.sync, nc.scalar]
            for cc in range(2):
                ps = [
                    psum_pool.tile([P, N], mybir.dt.float32, name=f"ps{cc}_{m}", tag="ps")
                    for m in range(M_T)
                ]
                for k in range(K_T):
                    tile_k, off = xwk[k]
                    rhs = tile_k[
                        :, off + ROWS_PER_CORE + cc * N : off + ROWS_PER_CORE + (cc + 1) * N
                    ]
                    for m in range(M_T):
                        nc.tensor.matmul(
                            ps[m][:],
                            tile_k[:, off + m * P : off + (m + 1) * P],
                            rhs,
                            start=(k == 0),
                            stop=(k == K_T - 1),
                        )
                for m in range(M_T):
                    zh = z_pool.tile([P, N], mybir.dt.float32, name=f"z{cc}_{m}", tag="z")
                    evict_eng[m % 2](zh[:], ps[m][:])
                    for r in range(n_rep):
                        out_eng[(m * n_rep + r) % 2].dma_start(
                            out_rep[m * P : (m + 1) * P, r, cc * N : (cc + 1) * N],
                            zh[:],
                        )

    _split_multiwaits(nc)
    return nc


_NC_CACHE: dict = {}


def _get_nc(compute_dt: str, out_mode: str) -> bass.Bass:
    key = (compute_dt, out_mode)
    if key not in _NC_CACHE:
        _NC_CACHE[key] = _build(compute_dt, out_mode)
    return _NC_CACHE[key]


def kernel(x_q, x_kv, Wq, Wk, Wv, Wproj, _compute_dt=None, _out_mode=None):
    compute_dt = _compute_dt or COMPUTE_DT
    out_mode = _out_mode or OUT_MODE
    B, Tkv, C_ = x_kv.shape
    assert (B, Tkv, C_) == (4, 2048, C)

    # Fold the two projections: z = x @ Wv.T @ Wproj.T = x @ WfT
    WfT = (Wv.astype(np.float64).T @ Wproj.astype(np.float64).T).astype(np.float32)

    x_flat = x_kv.reshape(B * Tkv, C)
    in_maps = []
    for c in range(N_CORES):
        shard = x_flat[c * ROWS_PER_CORE : (c + 1) * ROWS_PER_CORE]
        xw = np.concatenate([shard.T, WfT], axis=1)  # [C(k), 2048]
        if compute_dt == "bf16":
            import ml_dtypes

            xw = xw.astype(ml_dtypes.bfloat16)
        else:
            xw = np.ascontiguousarray(xw)
        in_maps.append({"xw": xw})

    nc = _get_nc(compute_dt, out_mode)
    res = run_bass_kernel_spmd(nc, in_maps, core_ids=list(range(N_CORES)))

    Tq = L * Tkv
    blocks = []
    for c in range(N_CORES):
        blk = res.results[c]["out"]
        if out_mode != "device3":
            blk = np.repeat(blk, L, axis=0)
        blocks.append(blk)
    out_flat = np.concatenate(blocks, axis=0)  # [B*Tq, C]
    return out_flat.reshape(B, Tq, C)



# revision 2
# speedup vs baseline: 1.4599x; 1.4599x over previous
"""Trainium2 Bass kernel for nn_CrossAttention_28183575396415.

The reference block-mask gives every query exactly one key (kv = q_idx // 3),
so the softmax weight is identically 1 and the q/k projections, RMSNorm and
RoPE are dead code.  The module reduces to

    out[b, t] = x_kv[b, t // 3] @ Wv.T @ Wproj.T
              = x_kv[b, t // 3] @ WfT          with WfT = Wv.T @ Wproj.T

Strategy (8 NeuronCores, SPMD, bf16 compute / fp32 PSUM):
  - Host folds the two projection matrices into WfT (float64 accumulate) and
    rounds x / WfT to bf16 (measured end-to-end rel_l2 ~3e-3 vs the 2e-2
    gate).  The 4*2048 = 8192 kv rows are row-sharded 8 ways; each core's
    shard is pre-transposed so the contraction dim lands on partitions:
        xw[:, :1024]  = x_shard.T
        xw[:, 1024:]  = WfT
  - Device: 8 row-sweeps of z = xT.T @ WfT, one 128-row m-tile each, full
    1024 output columns as 2 PSUM banks (2 x N=512 matmuls per k-tile).
    Four sweeps are in flight (8 PSUM banks); the k-loop is OUTER during
    the input stream so the PE consumes k-tiles in arrival order and never
    idles, then the remaining four sweeps run back-to-back.
  - ~3us of warmup matmuls on a zeroed scratch tile run during the ~8us
    framework prologue's tail so the PE clock is at 2.4 GHz (not the 1.2
    GHz cold pstate) when the first real k-tile lands.
  - Output: PSUM is evicted fp32->bf16 (vector engine does one 512-col
    half, scalar the other, in parallel), then ONE dma per sweep writes
    the t//3-replicated [128, 3, 1024] slice via a stride-0 broadcast
    source AP -- 8 output triggers total instead of 24.
  - Host unshard = concatenate the 8 [3072, 1024] slices, upcast to fp32.
"""

import json
import os

import numpy as np

import concourse.bass as bass
import concourse.mybir as mybir
from concourse.tile import TileContext
from concourse.vector_clock import ScopedClock
from concourse.bass_utils import run_bass_kernel_spmd

P = 128          # partitions
C = 1024         # model dim
K_T = C // P     # k tiles (contraction)
M_T = C // P     # row tiles per core shard
N = 512          # matmul free dim (one PSUM bank of fp32)
L = 3            # replication factor (Tq // Tkv)
ROWS_PER_CORE = 1024
N_CORES = 8
W2 = 2048        # per-k-tile free dim: [x_m 1024 | w 1024]

# knobs (A/B testing; defaults are the fast path)
OUT_DT = os.environ.get("KERNEL_OUT_DT", "bf16")     # "bf16" | "f32"
OUT_REP = os.environ.get("KERNEL_OUT_REP", "bcast")  # "bcast" | "multi"
WARMUP = int(os.environ.get("KERNEL_WARMUP", "8"))   # warmup matmuls
IN_SPLIT = os.environ.get("KERNEL_IN_SPLIT", "single")  # "single" | "pair"


class SlimTailTileContext(TileContext):
    """Tile's kernel tail is drain -> barrier -> ~280 serialized per-semaphore
    clear instructions -> barrier (~8 us measured).  The clears only matter if
    the loaded NEFF executes more than once; every kernel() call here builds a
    fresh jit executable (fresh NEFF load, semaphores re-initialized), so skip
    them and the second barrier.  The drain still waits for every DMA queue,
    so outputs are complete before the program ends."""

    def _drain_and_barrier(self, tick_clock, wait_clock):
        drain_inst = self.nc.sync.drain()
        wait_clock.add_sem_waits(
            drain_inst.ins, ScopedClock({None: tick_clock.global_clock})
        )
        popped = self.nc._tile_sem_poison_stack.pop()
        assert popped is self._sem_poison

def _split_multiwaits(nc: bass.Bass) -> None:
    """This container's walrus allows only ONE sync-wait on several
    instruction formats (Drain/CTRL, Matmult's LDWEIGHTS half, ...).  Tile
    can emit more.  Post-pass the serialized BIR: for any instruction with
    >1 on_wait, hoist all but the last wait onto single-wait EventSemaphore
    carriers inserted immediately before it on the same engine (waits then
    execute in queue order -- semantics unchanged)."""
    raw = bass.Bass.to_json_bytes(nc)
    j = json.loads(raw)
    for f in j["functions"]:
        for bb in f["blocks"]:
            new_insts = []
            for ins in bb["instructions"]:
                si = ins.get("sync_info")
                waits = si.get("on_wait", []) if si else []
                if len(waits) > 1:
                    for i, w in enumerate(waits[:-1]):
                        carrier = {
                            "engine": ins["engine"],
                            "ins": [],
                            "outs": [],
                            "name": f"{ins['name']}_hw{i}",
                            "opcode": "EventSemaphore",
                            "sync_info": {"on_update": [], "on_wait": [w]},
                        }
                        if "debug" in ins:
                            carrier["debug"] = ins["debug"]
                        new_insts.append(carrier)
                    si["on_wait"] = waits[-1:]
                new_insts.append(ins)
            bb["instructions"] = new_insts
    patched = json.dumps(j).encode()
    nc.to_json_bytes = lambda: patched


def _build(out_dt_s: str, rep_mode: str, warmup: int, in_split: str) -> bass.Bass:
    nc = bass.Bass("TRN2")
    in_dt = mybir.dt.bfloat16
    out_dt = mybir.dt.bfloat16 if out_dt_s == "bf16" else mybir.dt.float32
    f32 = mybir.dt.float32

    xw = nc.dram_tensor("xw", [C, W2], in_dt, kind="ExternalInput")
    out = nc.dram_tensor("out", [L * ROWS_PER_CORE, C], out_dt, kind="ExternalOutput")

    # out row (g*128 + p)*3 + r  <-  z row g*128 + p
    out_r = out.rearrange("(g p r) c -> g p r c", p=P, r=L)  # [8, 128, 3, 1024]

    with SlimTailTileContext(nc) as tc:
        with (
            tc.tile_pool(name="xw", bufs=1) as xw_pool,
            tc.tile_pool(name="warm", bufs=1) as warm_pool,
            tc.tile_pool(name="psum", bufs=8, space="PSUM") as psum_pool,
            tc.tile_pool(name="zout", bufs=4) as z_pool,
        ):
            # ---- PE warmup: ramp the clock out of the cold pstate during the
            # prologue tail, before the first k-tile arrives.  Zeroed scratch
            # (avoid NaN garbage), one PSUM bank, overwritten each time and
            # never read.
            if warmup:
                wt = warm_pool.tile([P, P + N], in_dt, name="warm", tag="warm")
                nc.vector.memset(wt[:], 0)
                wp = psum_pool.tile([P, N], f32, name="wps", tag="ps")
                for _ in range(warmup):
                    nc.tensor.matmul(wp[:], wt[:, :P], wt[:, P:], start=True, stop=True)

            # ---- input stream: all triggers on the sync engine so the single
            # HWDGE queue delivers k-tiles in order, one every ~1.3us.
            if in_split == "single":
                groups = [[k] for k in range(K_T)]
            else:
                groups = [[0], [1, 2], [3, 4], [5, 6], [7]]
            xwk = [None] * K_T
            for j, grp in enumerate(groups):
                n = len(grp)
                t = xw_pool.tile([P, n * W2], in_dt, name=f"xwp{j}", tag=f"xwp{j}")
                src = xw[grp[0] * P : (grp[0] + n) * P, :].rearrange(
                    "(g p) m -> p g m", p=P
                )
                dst = t[:].rearrange("p (g m) -> p g m", g=n)
                nc.sync.dma_start(dst, src)
                for i, k in enumerate(grp):
                    xwk[k] = (t, i * W2)

            ps: dict = {}

            def start_sweep(m):
                ps[m] = (
                    psum_pool.tile([P, N], f32, name=f"psa{m}", tag="ps"),
                    psum_pool.tile([P, N], f32, name=f"psb{m}", tag="ps"),
                )

            def mm(m, k):
                t, off = xwk[k]
                lhs = t[:, off + m * P : off + (m + 1) * P]
                s, e = (k == 0), (k == K_T - 1)
                nc.tensor.matmul(
                    ps[m][0][:], lhs, t[:, off + C : off + C + N], start=s, stop=e
                )
                nc.tensor.matmul(
                    ps[m][1][:], lhs, t[:, off + C + N : off + W2], start=s, stop=e
                )

            out_eng = [nc.sync, nc.scalar]

            def finish(m):
                z = z_pool.tile([P, C], out_dt, name=f"z{m}", tag="z")
                nc.vector.tensor_copy(z[:, :N], ps[m][0][:])
                nc.scalar.copy(z[:, N:], ps[m][1][:])
                dst = out_r[m]  # [128, 3, 1024]
                if rep_mode == "bcast":
                    src = z[:].unsqueeze(1).broadcast_to((P, L, C))
                    out_eng[m % 2].dma_start(dst, src)
                else:
                    for r in range(L):
                        out_eng[(m * L + r) % 2].dma_start(dst[:, r, :], z[:])

            # ---- phase A: first 4 sweeps, k-major (arrival order)
            for k in range(K_T):
                for m in range(4):
                    if k == 0:
                        start_sweep(m)
                    mm(m, k)
                    if k == K_T - 1:
                        finish(m)
            # ---- phase B: remaining 4 sweeps, sweep-major
            for m in range(4, M_T):
                start_sweep(m)
                for k in range(K_T):
                    mm(m, k)
                finish(m)

    _split_multiwaits(nc)
    return nc


_NC_CACHE: dict = {}


def _get_nc(*key) -> bass.Bass:
    if key not in _NC_CACHE:
        _NC_CACHE[key] = _build(*key)
    return _NC_CACHE[key]


def kernel(x_q, x_kv, Wq, Wk, Wv, Wproj,
           _out_dt=None, _out_rep=None, _warmup=None, _in_split=None):
    import ml_dtypes

    out_dt = _out_dt or OUT_DT
    rep_mode = _out_rep or OUT_REP
    warmup = WARMUP if _warmup is None else _warmup
    in_split = _in_split or IN_SPLIT
    B, Tkv, C_ = x_kv.shape
    assert (B, Tkv, C_) == (4, 2048, C)

    # Fold the two projections: z = x @ Wv.T @ Wproj.T = x @ WfT
    WfT = (Wv.astype(np.float64).T @ Wproj.astype(np.float64).T).astype(np.float32)

    x_flat = x_kv.reshape(B * Tkv, C)
    in_maps = []
    for c in range(N_CORES):
        shard = x_flat[c * ROWS_PER_CORE : (c + 1) * ROWS_PER_CORE]
        xw = np.concatenate([shard.T, WfT], axis=1)  # [C(k), 2048]
        in_maps.append({"xw": xw.astype(ml_dtypes.bfloat16)})

    nc = _get_nc(out_dt, rep_mode, warmup, in_split)
    res = run_bass_kernel_spmd(nc, in_maps, core_ids=list(range(N_CORES)))

    Tq = L * Tkv
    out_flat = np.concatenate([res.results[c]["out"] for c in range(N_CORES)], axis=0)
    return out_flat.astype(np.float32).reshape(B, Tq, C)
